# revision 2
# baseline (speedup 1.0000x reference)
"""Trainium2 Bass kernel for an MoE routing module.

Strategy: data-parallel over the batch — each of the 8 NeuronCores runs the
full pipeline (gating -> top-2 -> expert MLPs) for its 8 samples. All
data-dependent expert selection is done with indirect-DMA gathers driven by
index tiles computed on device; there are no collectives and no registers.

v2 (fp8 + transposed gathers):
  - gating: emb table is bf16; dma_gather(transpose=True) lands the tokens
    directly in [d-partition, s] layout, and pooling is a DVE free-axis
    reduce (mean folded into gate_w1 on host). The fp32 pooling matmuls and
    PE transposes of v1 are gone. Gate MLP stays fp32 so top-2 matches the
    fp32 reference (margin/noise ratio measured 5.4x for this seed).
  - experts: token embeddings and W1 are fp8 e4m3 scaled by 512 (descale
    folded into the relu activation's input scale). Tokens are gathered as
    fp8 rows (indirect DMA, int32 idx = e*V + x), then transposed on-chip by
    an SBUF-source dma_gather(transpose=True) with an identity int16 index
    permutation (token s lives at partition s%128, free stripe s//128).
    The transpose works at 16-bit granularity, so fp8 d-pairs interleave:
    tokT[p, jj, s, b] = tok_s[d = 2*(jj*128+p)+b]. W1 rows are permuted on
    the host to match, and the big [S,D]@[D,H] matmul runs in DoubleRow fp8
    perf mode (2 k-subtiles per pass): lhsT [128, 2, 128] x rhs [128, 2, 512].
  - W2 is stored bf16 hi+lo (reconstructs ~fp32 on device; bf16-only W2
    costs 1.7e-3 rel err). Whole-pipeline numpy sim: rel err ~3.9e-3.

HW gotcha (verified on device): indirect DMA consumes exactly ONE index per
destination partition — multi-index-per-partition gathers return garbage.
"""

import os
import sys

for _p in ("/opt/trn_rl_repo", "/root/.axon_site/_ro/trn_rl_repo"):
    if os.path.isdir(_p) and _p not in sys.path:
        sys.path.insert(0, _p)

import numpy as np

import concourse.bacc as bacc
import concourse.tile as tile
import concourse.mybir as mybir
from concourse.bass import IndirectOffsetOnAxis
from concourse.bass_utils import run_bass_kernel_spmd
from concourse.masks import make_identity

F32 = mybir.dt.float32
BF16 = mybir.dt.bfloat16
F8 = mybir.dt.float8e4
I32 = mybir.dt.int32
I16 = mybir.dt.int16
U32 = mybir.dt.uint32

V, D, H, E, C, TOPK = 16000, 1024, 1024, 8, 16, 2
B, S = 64, 512
GATE_H = 256
NCORES = 8
BL = B // NCORES          # samples per core
DT = D // 128             # 8 d-tiles
HT = H // 128             # 8 h-tiles
ST = S // 128             # 4 s-tiles
MT = GATE_H // 128        # 2 gate-hidden tiles
NGRP = 2                  # sample groups per core (pipelining)
GBL = BL // NGRP          # samples per group

FP8_SCALE = 512.0         # exp_emb/exp_w1 host-side scale into e4m3 range

# small bf16 weight table columns (per-expert W2 hi/lo + biases)
W2COL = 0                 # W2 hi  (HT*C = 128 cols)
W2LO = W2COL + HT * C     # 128    W2 lo
B1COL = W2LO + HT * C     # 256    b1 (HT cols)
B2COL = B1COL + HT        # 264    b2 (1 col, partitions 0..C-1)
WRCOLS = 272              # padded row length

_compiled = {}
last_results = None       # BassKernelResults of the most recent run (for test.py)


def build_program(reps=1):
    """reps>1 repeats the whole compute body (benchmarking aid)."""
    nc = bacc.Bacc("TRN2", target_bir_lowering=False, debug=False, num_devices=NCORES)
    act = mybir.ActivationFunctionType

    x_t = nc.dram_tensor("x_loc", [BL, S], I32, kind="ExternalInput")
    xw_t = nc.dram_tensor("xw16", [128, BL, S // 16], I16, kind="ExternalInput")
    iw_t = nc.dram_tensor("iw16", [128, S // 16], I16, kind="ExternalInput")
    emb_t = nc.dram_tensor("emb", [V, D], BF16, kind="ExternalInput")
    eemb_t = nc.dram_tensor("eemb", [E * V, D], F8, kind="ExternalInput")
    wf8_t = nc.dram_tensor("wf8", [E * 128, DT * H], F8, kind="ExternalInput")
    wallr_t = nc.dram_tensor("wallr", [E * 128, WRCOLS], BF16, kind="ExternalInput")
    gw1_t = nc.dram_tensor("gw1", [D, GATE_H], F32, kind="ExternalInput")
    gb1_t = nc.dram_tensor("gb1", [128, MT], F32, kind="ExternalInput")
    gw2_t = nc.dram_tensor("gw2", [GATE_H, E], F32, kind="ExternalInput")
    gb2_t = nc.dram_tensor("gb2", [E, 1], F32, kind="ExternalInput")
    out_t = nc.dram_tensor("out", [BL, C], F32, kind="ExternalOutput")

    with tile.TileContext(nc) as tc:
        with (
            tc.tile_pool(name="const", bufs=1) as cpool,
            tc.tile_pool(name="dram", bufs=1, space="DRAM") as dpool,
        ):
            # ---- constants ----
            id_f = cpool.tile([128, 128], F32)
            make_identity(nc, id_f[:, :])
            ones_m = cpool.tile([1, 128], F32)      # lhsT for K=1 broadcast MMs
            nc.vector.memset(ones_m[:, :], 1.0)
            iota_p = cpool.tile([128, 1], I32)      # value = partition index
            nc.gpsimd.iota(iota_p[:, :], pattern=[[0, 1]], base=0, channel_multiplier=1)

            # token ids, transposed: xt[p, b, t] = x[b, t*128+p]
            xt = cpool.tile([128, BL, ST], I32)
            nc.sync.dma_start(
                out=xt[:, :, :], in_=x_t[:, :].rearrange("b (t p) -> p b t", p=128)
            )
            # int16 wrapped indices for dma_gather (pre-wrapped on host)
            xw = cpool.tile([128, BL, S // 16], I16)
            nc.sync.dma_start(out=xw[:, :, :], in_=xw_t[:, :, :])
            # identity permutation (wrapped) for the SBUF-source transpose gather
            iw = cpool.tile([128, S // 16], I16)
            nc.sync.dma_start(out=iw[:, :], in_=iw_t[:, :])

            gb1_sb = cpool.tile([128, MT], F32)
            nc.sync.dma_start(out=gb1_sb[:, :], in_=gb1_t[:, :])
            gb2_sb = cpool.tile([E, 1], F32)
            nc.sync.dma_start(out=gb2_sb[:, :], in_=gb2_t[:, :])
            gw1_sb = cpool.tile([128, DT, GATE_H], F32)
            nc.sync.dma_start(
                out=gw1_sb[:, :, :], in_=gw1_t[:, :].rearrange("(j p) g -> p j g", p=128)
            )
            gw2_sb = cpool.tile([128, MT, E], F32)
            nc.sync.dma_start(
                out=gw2_sb[:, :, :], in_=gw2_t[:, :].rearrange("(m p) e -> p m e", p=128)
            )

            consts = dict(
                id_f=id_f, ones_m=ones_m, iota_p=iota_p, xt=xt, xw=xw, iw=iw,
                gb1_sb=gb1_sb, gb2_sb=gb2_sb, gw1_sb=gw1_sb, gw2_sb=gw2_sb,
            )
            tensors = dict(
                emb_t=emb_t, eemb_t=eemb_t, wf8_t=wf8_t, wallr_t=wallr_t, out_t=out_t,
            )
            # chain tile serializes reps so the benchmark differential is honest
            chain = None
            if reps > 1:
                chain = cpool.tile([1, 1], F32)
                nc.vector.memset(chain[:, :], 0.0)
            for rep in range(reps):
                _body_once(nc, tc, act, rep, dpool, consts, tensors, chain)

    nc.compile()
    return nc


def _body_once(nc, tc, act, rep, dpool, cn, tn, chain=None):
    sfx = f"_r{rep}"
    id_f = cn["id_f"]
    ones_m, iota_p = cn["ones_m"], cn["iota_p"]
    xt, xw, iw = cn["xt"], cn["xw"], cn["iw"]
    gb1_sb, gb2_sb, gw1_sb, gw2_sb = cn["gb1_sb"], cn["gb2_sb"], cn["gw1_sb"], cn["gw2_sb"]
    emb_t, eemb_t, wf8_t, wallr_t, out_t = (
        tn["emb_t"], tn["eemb_t"], tn["wf8_t"], tn["wallr_t"], tn["out_t"],
    )
    inv_s2 = 1.0 / (FP8_SCALE * FP8_SCALE)

    with (
        tc.tile_pool(name=f"persist{sfx}", bufs=1) as ppool,
        tc.tile_pool(name=f"bc{sfx}", bufs=2) as bcpool,
        # gating pools
        tc.tile_pool(name=f"gat{sfx}", bufs=2) as gpool,
        tc.tile_pool(name=f"gsb{sfx}", bufs=2) as gspool,
        tc.tile_pool(name=f"gpss{sfx}", bufs=2, space="PSUM") as gps_s,
        # expert pools
        tc.tile_pool(name=f"exi{sfx}", bufs=3) as xipool,
        tc.tile_pool(name=f"etok{sfx}", bufs=3) as tokpool,
        tc.tile_pool(name=f"ew{sfx}", bufs=2) as wpool,
        tc.tile_pool(name=f"ewr{sfx}", bufs=2) as wrpool,
        tc.tile_pool(name=f"ett{sfx}", bufs=3) as ttpool,
        tc.tile_pool(name=f"esm{sfx}", bufs=3) as smpool,
        tc.tile_pool(name=f"ejunk{sfx}", bufs=2) as junkpool,
        tc.tile_pool(name=f"epsz{sfx}", bufs=2, space="PSUM") as eps_z,
        tc.tile_pool(name=f"epso{sfx}", bufs=1, space="PSUM") as eps_o,
    ):
        out_acc = ppool.tile([C, BL], F32)
        nc.vector.memset(out_acc[:, :], 0.0)

        for g in range(NGRP):
            b0 = g * GBL
            # ============ gating for samples [b0, b0+GBL) ============
            # pooled^T accumulated straight into [d-partition, j, sample]
            pts = gspool.tile([128, DT, GBL], F32, tag="pts")
            for bl in range(GBL):
                b = b0 + bl
                gtokT = gpool.tile([128, DT, S], BF16, tag="gtokT")
                nc.gpsimd.dma_gather(
                    out_ap=gtokT[:, :, :],
                    in_ap=emb_t[:, :],
                    idxs_ap=xw[:, b, :],
                    num_idxs=S,
                    num_idxs_reg=S,
                    elem_size=D,
                    transpose=True,
                )
                # sum over s (mean folded into gate_w1 host-side)
                nc.vector.tensor_reduce(
                    out=pts[:, :, bl : bl + 1].rearrange("p j o -> p (j o)"),
                    in_=gtokT[:, :, :],
                    axis=mybir.AxisListType.X,
                    op=mybir.AluOpType.add,
                )

            # gate layer 1 + relu
            hR = gspool.tile([128, MT, GBL], F32, tag="hR")
            for m in range(MT):
                h_ps = gps_s.tile([128, GBL], F32, tag="gmisc")
                for j in range(DT):
                    nc.tensor.matmul(
                        out=h_ps[:, :],
                        lhsT=gw1_sb[:, j, m * 128 : (m + 1) * 128],
                        rhs=pts[:, j, :],
                        start=(j == 0),
                        stop=(j == DT - 1),
                    )
                nc.scalar.activation(
                    out=hR[:, m, :],
                    in_=h_ps[:, :],
                    func=act.Relu,
                    bias=gb1_sb[:, m : m + 1],
                )

            # gate layer 2 -> logits [e, b]
            l_ps = gps_s.tile([E, GBL], F32, tag="gmisc")
            for m in range(MT):
                nc.tensor.matmul(
                    out=l_ps[:, :],
                    lhsT=gw2_sb[:, m, :],
                    rhs=hR[:, m, :],
                    start=(m == 0),
                    stop=(m == MT - 1),
                )
            l_sb = gspool.tile([E, GBL], F32, tag="l_sb")
            nc.scalar.activation(
                out=l_sb[:, :], in_=l_ps[:, :], func=act.Identity,
                bias=gb2_sb[:, 0:1],
            )
            # transpose logits -> [b, e]
            lt_ps = gps_s.tile([GBL, E], F32, tag="gmisc")
            nc.tensor.matmul(
                out=lt_ps[:, :], lhsT=l_sb[:, :], rhs=id_f[0:E, 0:E],
                start=True, stop=True,
            )
            lt_sb = gspool.tile([GBL, E], F32, tag="lt_sb")
            nc.vector.tensor_copy(lt_sb[:, :], lt_ps[:, :])

            # top-2 of logits == top-2 of softmax (monotone)
            mx = gspool.tile([GBL, 8], F32, tag="mx")
            mi = gspool.tile([GBL, 8], U32, tag="mi")
            nc.vector.max_with_indices(mx[:, :], mi[:, :], lt_sb[:, :])

            # renormalized top-2 softmax weights:
            # rw1 = 1/(1+exp(l2-l1)), rw2 = exp(l2-l1)/(1+exp(l2-l1))
            dlt = gspool.tile([GBL, 1], F32, tag="dlt")
            nc.vector.tensor_sub(dlt[:, :], mx[:, 1:2], mx[:, 0:1])
            q = gspool.tile([GBL, 1], F32, tag="q")
            nc.scalar.activation(out=q[:, :], in_=dlt[:, :], func=act.Exp)
            sden = gspool.tile([GBL, 1], F32, tag="sden")
            nc.vector.tensor_scalar_add(sden[:, :], q[:, :], 1.0)
            rw1 = gspool.tile([GBL, 1], F32, tag="rw1")
            nc.vector.reciprocal(rw1[:, :], sden[:, :])
            rw2 = gspool.tile([GBL, 1], F32, tag="rw2")
            nc.vector.tensor_mul(rw2[:, :], q[:, :], rw1[:, :])

            # pack per-(b,k) scalars: cols bl*8 + {0,1}=e*V, {2,3}=e*128,
            # {6,7}=rw ({4,5} unused)
            ei_f = gspool.tile([GBL, TOPK], F32, tag="ei_f")
            nc.vector.tensor_copy(ei_f[:, :], mi[:, 0:TOPK])
            vals = gspool.tile([GBL, 8], F32, tag="vals")
            nc.vector.tensor_scalar_mul(vals[:, 0:2], ei_f[:, :], float(V))
            nc.vector.tensor_scalar_mul(vals[:, 2:4], ei_f[:, :], 128.0)
            nc.vector.tensor_scalar_mul(vals[:, 4:6], ei_f[:, :], 0.0)
            nc.vector.tensor_copy(vals[:, 6:7], rw1[:, :])
            nc.vector.tensor_copy(vals[:, 7:8], rw2[:, :])

            # broadcast across partitions: bounce through DRAM to get a flat
            # [1, GBL*8] row, then K=1 matmul against ones.
            scratch = dpool.tile([GBL, 8], F32, tag=f"scratch{sfx}_{g}")
            nc.sync.dma_start(out=scratch[:, :], in_=vals[:, :])
            if chain is not None:
                # unused col 4: forces rep r to wait on rep r-1's result
                nc.sync.dma_start(out=scratch[0:1, 4:5], in_=chain[0:1, 0:1])
            flat = gspool.tile([1, GBL * 8], F32, tag="flat")
            nc.sync.dma_start(
                out=flat[0:1, :].rearrange("p (b c) -> p b c", b=GBL),
                in_=scratch[:, :],
            )
            bc_ps = gps_s.tile([128, GBL * 8], F32, tag="gmisc")
            nc.tensor.matmul(
                out=bc_ps[:, :], lhsT=ones_m[:, :], rhs=flat[0:1, :],
                start=True, stop=True,
            )
            BCf = bcpool.tile([128, GBL * 8], F32, tag="bcf")
            BCi = bcpool.tile([128, GBL * 8], I32, tag="bci")
            nc.vector.tensor_copy(BCf[:, :], bc_ps[:, :])
            nc.vector.tensor_copy(BCi[:, :], bc_ps[:, :])  # cast f32->i32

            # ============ experts for this group (fp8 DoubleRow) ============
            for bl in range(GBL):
                b = b0 + bl
                for k in range(TOPK):
                    cEV = bl * 8 + k
                    cE128 = bl * 8 + 2 + k
                    cRW = bl * 8 + 6 + k

                    tok_idx = xipool.tile([128, ST], I32, tag="tok_idx")
                    nc.vector.tensor_add(
                        tok_idx[:, :],
                        xt[:, b, :],
                        BCi[:, cEV : cEV + 1].to_broadcast([128, ST]),
                    )
                    w_idx = xipool.tile([128, 1], I32, tag="w_idx")
                    nc.vector.tensor_add(
                        w_idx[:, :], iota_p[:, :], BCi[:, cE128 : cE128 + 1]
                    )

                    # fp8 token rows: token s -> partition s%128, stripe s//128
                    tok8 = tokpool.tile([128, ST, D], F8, tag="tok8")
                    for t in range(ST):
                        nc.gpsimd.indirect_dma_start(
                            out=tok8[:, t, :],
                            out_offset=None,
                            in_=eemb_t[:, :],
                            in_offset=IndirectOffsetOnAxis(
                                ap=tok_idx[:, t : t + 1], axis=0
                            ),
                        )
                    # per-expert weights: W1 (fp8, row-permuted) + small bf16 row
                    wg1 = wpool.tile([128, DT * H], F8, tag="wg1")
                    nc.gpsimd.indirect_dma_start(
                        out=wg1[:, :],
                        out_offset=None,
                        in_=wf8_t[:, :],
                        in_offset=IndirectOffsetOnAxis(ap=w_idx[:, :], axis=0),
                    )
                    wgr = wrpool.tile([128, WRCOLS], BF16, tag="wgr")
                    nc.gpsimd.indirect_dma_start(
                        out=wgr[:, :],
                        out_offset=None,
                        in_=wallr_t[:, :],
                        in_offset=IndirectOffsetOnAxis(ap=w_idx[:, :], axis=0),
                    )
                    b1f = smpool.tile([128, HT], F32, tag="b1f")
                    nc.vector.tensor_copy(b1f[:, :], wgr[:, B1COL : B1COL + HT])
                    b2f = smpool.tile([C, 1], F32, tag="b2f")
                    nc.vector.tensor_copy(b2f[:, :], wgr[0:C, B2COL : B2COL + 1])
                    w2f = smpool.tile([128, HT * C], F32, tag="w2f")
                    nc.vector.tensor_add(
                        w2f[:, :], wgr[:, W2COL : W2COL + HT * C],
                        wgr[:, W2LO : W2LO + HT * C],
                    )

                    # on-chip transpose: SBUF-source gather, identity perm.
                    # 16-bit granularity: tokT[p, jj, 2s+b] = tok_s[2*(jj*128+p)+b]
                    tokT = ttpool.tile([128, DT, S], F8, tag="tokT")
                    nc.gpsimd.dma_gather(
                        out_ap=tokT[:, :, :],
                        in_ap=tok8[:, :, :].rearrange("p t d -> p (t d)"),
                        idxs_ap=iw[:, :],
                        num_idxs=S,
                        num_idxs_reg=S,
                        elem_size=D,
                        transpose=True,
                        sbuf_tokens_per_rank=128,
                        sbuf_free_dim_per_rank=D,
                    )
                    # view as [p, jj(4), s(512), b(2)]
                    tview = tokT[:, :, :].rearrange("p a s -> p (a s)").rearrange(
                        "p (j s b) -> p j s b", j=DT // 2, b=2
                    )
                    wview = wg1[:, :].rearrange("p (a h) -> p a h", a=DT)

                    # z[h_tile] = relu((tokT.T @ W1) / SCALE^2 + b1); sum over s
                    pacc = smpool.tile([128, HT], F32, tag="pacc")
                    for j2 in range(HT):
                        z_ps = eps_z.tile([128, S], F32, tag="z")
                        mm = 0
                        for bb in range(2):
                            for jjp in (0, 2):
                                nc.tensor.matmul(
                                    out=z_ps[:, :],
                                    lhsT=wview[
                                        :, bb * 4 + jjp : bb * 4 + jjp + 2,
                                        j2 * 128 : (j2 + 1) * 128,
                                    ],
                                    rhs=tview[:, jjp : jjp + 2, :, bb : bb + 1],
                                    start=(mm == 0),
                                    stop=(mm == 3),
                                    perf_mode=mybir.MatmulPerfMode.DoubleRow,
                                )
                                mm += 1
                        zjunk = junkpool.tile([128, S], BF16, tag="zjunk")
                        nc.scalar.activation(
                            out=zjunk[:, :],
                            in_=z_ps[:, :],
                            func=act.Relu,
                            scale=inv_s2,
                            bias=b1f[:, j2 : j2 + 1],
                            accum_out=pacc[:, j2 : j2 + 1],
                        )

                    psc = smpool.tile([128, HT], F32, tag="psc")
                    nc.vector.tensor_scalar_mul(psc[:, :], pacc[:, :], 1.0 / S)

                    eo_ps = eps_o.tile([C, 1], F32, tag="eo")
                    for j2 in range(HT):
                        nc.tensor.matmul(
                            out=eo_ps[:, :],
                            lhsT=w2f[:, j2 * C : (j2 + 1) * C],
                            rhs=psc[:, j2 : j2 + 1],
                            start=(j2 == 0),
                            stop=(j2 == HT - 1),
                        )
                    eo1 = smpool.tile([C, 1], F32, tag="eo1")
                    nc.scalar.activation(
                        out=eo1[:, :], in_=eo_ps[:, :], func=act.Identity,
                        bias=b2f[:, 0:1],
                    )
                    eo2 = smpool.tile([C, 1], F32, tag="eo2")
                    nc.vector.tensor_mul(eo2[:, :], eo1[:, :], BCf[0:C, cRW : cRW + 1])
                    nc.vector.tensor_add(
                        out_acc[:, b : b + 1], out_acc[:, b : b + 1], eo2[:, :]
                    )

        if chain is not None:
            nc.vector.tensor_copy(chain[0:1, 0:1], out_acc[0:1, 0:1])
        nc.sync.dma_start(
            out=out_t[:, :].rearrange("b c -> c b"), in_=out_acc[:, :]
        )


def _wrap16(idx):
    """Wrap a [N] index vector into the [128, N//16] int16 layout the Q7
    dma_gather ucode expects (16 partitions, replicated 8x)."""
    n = idx.shape[0]
    w = idx.reshape(n // 16, 16).T.astype(np.int16)   # [16, N//16]
    return np.tile(w, (8, 1))                         # [128, N//16]


def _prep_inputs(inputs):
    """Host-side dtype casts + re-layouts shared by all cores."""
    import ml_dtypes

    f32 = np.float32
    bf16 = ml_dtypes.bfloat16
    f8 = ml_dtypes.float8_e4m3

    x = np.asarray(inputs["x"]).astype(np.int32)
    # int16 indices wrapped for dma_gather: xw16[16g+p, b, c] = x[b, c*16+p]
    xw = x.reshape(B, S // 16, 16).transpose(2, 0, 1).astype(np.int16)  # [16, B, 32]
    xw16 = np.tile(xw, (8, 1, 1))                                       # [128, B, 32]
    iw16 = _wrap16(np.arange(S))                                        # [128, 32]

    emb = np.asarray(inputs["emb"], dtype=f32).astype(bf16)

    def to_f8(a):
        return np.clip(a * FP8_SCALE, -240.0, 240.0).astype(f8)

    exp_emb = to_f8(
        np.ascontiguousarray(np.asarray(inputs["exp_emb"], dtype=f32).reshape(E * V, D))
    )

    # W1: fp8, rows permuted so wf8[e*128+p, (b*4+jj)*H + h] = W1[e, 2*(jj*128+p)+b, h]
    w1 = np.asarray(inputs["exp_w1"], dtype=f32)          # [E, D, H]
    # d = 2*(jj*128 + p) + b  ->  index [jj, p, b]
    w1p = w1.reshape(E, DT // 2, 128, 2, H)               # [E, jj, p, b, H]
    w1p = w1p.transpose(0, 2, 3, 1, 4)                    # [E, p, b, jj, H]
    wf8 = to_f8(np.ascontiguousarray(w1p.reshape(E * 128, DT * H)))

    w2 = np.asarray(inputs["exp_w2"], dtype=f32)          # [E, H, C]
    ew2 = w2.reshape(E, HT, 128, C).transpose(0, 2, 1, 3).reshape(E * 128, HT * C)
    b1 = np.asarray(inputs["exp_b1"], dtype=f32)          # [E, H]
    b1r = b1.reshape(E, HT, 128).transpose(0, 2, 1).reshape(E * 128, HT)
    b2 = np.asarray(inputs["exp_b2"], dtype=f32)          # [E, C]
    b2slot = np.zeros((E * 128, 1), f32)
    for e in range(E):
        b2slot[e * 128 : e * 128 + C, 0] = b2[e]
    w2hi = ew2.astype(bf16).astype(f32)
    w2lo = ew2 - w2hi
    wallr = np.zeros((E * 128, WRCOLS), f32)
    wallr[:, W2COL : W2COL + HT * C] = w2hi
    wallr[:, W2LO : W2LO + HT * C] = w2lo
    wallr[:, B1COL : B1COL + HT] = b1r
    wallr[:, B2COL : B2COL + 1] = b2slot
    wallr = np.ascontiguousarray(wallr).astype(bf16)

    # mean over S folded into gate_w1
    gw1 = np.ascontiguousarray(np.asarray(inputs["gate_w1"], dtype=f32) / S)
    gb1 = np.ascontiguousarray(
        np.asarray(inputs["gate_b1"], dtype=f32).reshape(MT, 128).T
    )
    gw2 = np.ascontiguousarray(np.asarray(inputs["gate_w2"], dtype=f32))
    gb2 = np.ascontiguousarray(np.asarray(inputs["gate_b2"], dtype=f32).reshape(E, 1))

    shared = dict(
        iw16=iw16, emb=emb, eemb=exp_emb, wf8=wf8, wallr=wallr,
        gw1=gw1, gb1=gb1, gw2=gw2, gb2=gb2,
    )
    return x, xw16, shared


def kernel(**inputs) -> np.ndarray:
    global last_results
    if "nc" not in _compiled:
        _compiled["nc"] = build_program()
    nc = _compiled["nc"]

    x, xw16, shared = _prep_inputs(inputs)
    in_maps = [
        {
            "x_loc": np.ascontiguousarray(x[c * BL : (c + 1) * BL]),
            "xw16": np.ascontiguousarray(xw16[:, c * BL : (c + 1) * BL]),
            **shared,
        }
        for c in range(NCORES)
    ]
    res = run_bass_kernel_spmd(
        nc, in_maps, list(range(NCORES)),
        trace=os.environ.get("KERNEL_TRACE", "0") == "1",
    )
    last_results = res
    out = np.concatenate([res.results[c]["out"] for c in range(NCORES)], axis=0)
    return np.ascontiguousarray(out.astype(np.float32))


# revision 3
# speedup vs baseline: 1.1037x; 1.1037x over previous
"""Trainium2 Bass kernel for an MoE routing module.

Strategy: data-parallel over the batch — each of the 8 NeuronCores runs the
full pipeline (gating -> top-2 -> expert MLPs) for its 8 samples. All
data-dependent expert selection is done with indirect-DMA gathers driven by
index tiles computed on device; there are no collectives and no registers.

v3 (fp8 DoubleRow + cheap gating):
  - gating: emb table is bf16; dma_gather(transpose=True) lands the tokens
    directly in [d-partition, s] layout, and pooling is a DVE free-axis
    reduce (mean folded into gate_w1 on host). Gate MLP stays fp32 so top-2
    matches the fp32 reference (margin/noise ratio measured 5.4x this seed).
  - experts: token embeddings gathered bf16 (pre-scaled x512 on host),
    PE-transposed against identity into fp32 PSUM, and the PSUM->SBUF copy
    casts to fp8 e4m3 (scaled values fit e4m3 range). W1 is fp8 (x512).
    The big [S,D]@[D,H] matmul runs DoubleRow fp8 (2 k-subtiles per pass,
    production AP layout: lhsT [128, 2t:2t+2, h], rhs tokT[:, 2t:2t+2, :],
    contiguous columns). Descale by 1/512^2 is folded into the b1 bias
    (host: b1 *= 512^2) and the psc mean multiply, so relu+s-pooling stays
    one activation op with accum_out per h-tile.
  - W2 is stored bf16 hi+lo and applied as 16 accumulating bf16 N=1 matmuls
    against a bf16 psc (fp32 4-pass matmuls were ~2x slower).
    Whole-pipeline numpy sim: rel err ~3.9e-3 (threshold 2e-2).

HW gotchas (verified on device):
  - indirect DMA consumes exactly ONE index per destination partition —
    multi-index-per-partition gathers return garbage.
  - DoubleRow rhs columns must be contiguous (stride-2 fp8 column APs halve
    the stream rate: ~460ns vs ~240ns per MM).
  - Q7/SWDGE descriptor generation is serial: ~1.1us per 128-row indirect
    gather, ~4.8us per 512-token dma_gather ucode call. Budget it.
"""

import os
import sys

for _p in ("/opt/trn_rl_repo", "/root/.axon_site/_ro/trn_rl_repo"):
    if os.path.isdir(_p) and _p not in sys.path:
        sys.path.insert(0, _p)

import numpy as np

import concourse.bacc as bacc
import concourse.tile as tile
import concourse.mybir as mybir
from concourse.bass import IndirectOffsetOnAxis
from concourse.bass_utils import run_bass_kernel_spmd
from concourse.masks import make_identity

F32 = mybir.dt.float32
BF16 = mybir.dt.bfloat16
F8 = mybir.dt.float8e4
I32 = mybir.dt.int32
I16 = mybir.dt.int16
U32 = mybir.dt.uint32

V, D, H, E, C, TOPK = 16000, 1024, 1024, 8, 16, 2
B, S = 64, 512
GATE_H = 256
NCORES = 8
BL = B // NCORES          # samples per core
DT = D // 128             # 8 d-tiles
HT = H // 128             # 8 h-tiles
ST = S // 128             # 4 s-tiles
MT = GATE_H // 128        # 2 gate-hidden tiles
NGRP = 2                  # sample groups per core (pipelining)
GBL = BL // NGRP          # samples per group

FP8_SCALE = 512.0         # exp_emb/exp_w1 host-side scale into e4m3 range

# small bf16 weight table columns (per-expert W2 hi/lo + biases)
W2COL = 0                 # W2 hi  (HT*C = 128 cols)
W2LO = W2COL + HT * C     # 128    W2 lo
B1COL = W2LO + HT * C     # 256    b1 * FP8_SCALE^2 (HT cols)
B2COL = B1COL + HT        # 264    b2 (1 col, partitions 0..C-1)
WRCOLS = 272              # padded row length

_compiled = {}
last_results = None       # BassKernelResults of the most recent run (for test.py)


def build_program(reps=1):
    """reps>1 repeats the whole compute body (benchmarking aid)."""
    nc = bacc.Bacc("TRN2", target_bir_lowering=False, debug=False, num_devices=NCORES)
    act = mybir.ActivationFunctionType

    x_t = nc.dram_tensor("x_loc", [BL, S], I32, kind="ExternalInput")
    xw_t = nc.dram_tensor("xw16", [128, BL, S // 16], I16, kind="ExternalInput")
    emb_t = nc.dram_tensor("emb", [V, D], BF16, kind="ExternalInput")
    eemb_t = nc.dram_tensor("eemb", [E * V, D], BF16, kind="ExternalInput")
    wf8_t = nc.dram_tensor("wf8", [E * 128, DT * H], F8, kind="ExternalInput")
    wallr_t = nc.dram_tensor("wallr", [E * 128, WRCOLS], BF16, kind="ExternalInput")
    gw1_t = nc.dram_tensor("gw1", [D, GATE_H], F32, kind="ExternalInput")
    gb1_t = nc.dram_tensor("gb1", [128, MT], F32, kind="ExternalInput")
    gw2_t = nc.dram_tensor("gw2", [GATE_H, E], F32, kind="ExternalInput")
    gb2_t = nc.dram_tensor("gb2", [E, 1], F32, kind="ExternalInput")
    out_t = nc.dram_tensor("out", [BL, C], F32, kind="ExternalOutput")

    with tile.TileContext(nc) as tc:
        with (
            tc.tile_pool(name="const", bufs=1) as cpool,
            tc.tile_pool(name="dram", bufs=1, space="DRAM") as dpool,
        ):
            # ---- constants ----
            id_bf = cpool.tile([128, 128], BF16)
            make_identity(nc, id_bf[:, :])
            id_f = cpool.tile([128, 128], F32)
            make_identity(nc, id_f[:, :])
            ones_m = cpool.tile([1, 128], F32)      # lhsT for K=1 broadcast MMs
            nc.vector.memset(ones_m[:, :], 1.0)
            iota_p = cpool.tile([128, 1], I32)      # value = partition index
            nc.gpsimd.iota(iota_p[:, :], pattern=[[0, 1]], base=0, channel_multiplier=1)

            # token ids, transposed: xt[p, b, t] = x[b, t*128+p]
            xt = cpool.tile([128, BL, ST], I32)
            nc.sync.dma_start(
                out=xt[:, :, :], in_=x_t[:, :].rearrange("b (t p) -> p b t", p=128)
            )
            # int16 wrapped indices for dma_gather (pre-wrapped on host)
            xw = cpool.tile([128, BL, S // 16], I16)
            nc.sync.dma_start(out=xw[:, :, :], in_=xw_t[:, :, :])

            gb1_sb = cpool.tile([128, MT], F32)
            nc.sync.dma_start(out=gb1_sb[:, :], in_=gb1_t[:, :])
            gb2_sb = cpool.tile([E, 1], F32)
            nc.sync.dma_start(out=gb2_sb[:, :], in_=gb2_t[:, :])
            gw1_sb = cpool.tile([128, DT, GATE_H], F32)
            nc.sync.dma_start(
                out=gw1_sb[:, :, :], in_=gw1_t[:, :].rearrange("(j p) g -> p j g", p=128)
            )
            gw2_sb = cpool.tile([128, MT, E], F32)
            nc.sync.dma_start(
                out=gw2_sb[:, :, :], in_=gw2_t[:, :].rearrange("(m p) e -> p m e", p=128)
            )

            consts = dict(
                id_bf=id_bf, id_f=id_f, ones_m=ones_m, iota_p=iota_p, xt=xt, xw=xw,
                gb1_sb=gb1_sb, gb2_sb=gb2_sb, gw1_sb=gw1_sb, gw2_sb=gw2_sb,
            )
            tensors = dict(
                emb_t=emb_t, eemb_t=eemb_t, wf8_t=wf8_t, wallr_t=wallr_t, out_t=out_t,
            )
            # chain tile serializes reps so the benchmark differential is honest
            chain = None
            if reps > 1:
                chain = cpool.tile([1, 1], F32)
                nc.vector.memset(chain[:, :], 0.0)
            for rep in range(reps):
                _body_once(nc, tc, act, rep, dpool, consts, tensors, chain)

    nc.compile()
    return nc


def _body_once(nc, tc, act, rep, dpool, cn, tn, chain=None):
    sfx = f"_r{rep}"
    id_bf, id_f = cn["id_bf"], cn["id_f"]
    ones_m, iota_p = cn["ones_m"], cn["iota_p"]
    xt, xw = cn["xt"], cn["xw"]
    gb1_sb, gb2_sb, gw1_sb, gw2_sb = cn["gb1_sb"], cn["gb2_sb"], cn["gw1_sb"], cn["gw2_sb"]
    emb_t, eemb_t, wf8_t, wallr_t, out_t = (
        tn["emb_t"], tn["eemb_t"], tn["wf8_t"], tn["wallr_t"], tn["out_t"],
    )
    # p = pacc * inv_s2 / S  (descale folded here + into host-scaled b1)
    psc_scale = 1.0 / (FP8_SCALE * FP8_SCALE * S)

    with (
        tc.tile_pool(name=f"persist{sfx}", bufs=1) as ppool,
        tc.tile_pool(name=f"bc{sfx}", bufs=2) as bcpool,
        # gating pools
        tc.tile_pool(name=f"gat{sfx}", bufs=2) as gpool,
        tc.tile_pool(name=f"gsb{sfx}", bufs=2) as gspool,
        tc.tile_pool(name=f"gpss{sfx}", bufs=2, space="PSUM") as gps_s,
        # expert pools
        tc.tile_pool(name=f"exi{sfx}", bufs=3) as xipool,
        tc.tile_pool(name=f"etok{sfx}", bufs=3) as tokpool,
        tc.tile_pool(name=f"ew{sfx}", bufs=2) as wpool,
        tc.tile_pool(name=f"ewr{sfx}", bufs=2) as wrpool,
        tc.tile_pool(name=f"ett{sfx}", bufs=2) as ttpool,
        tc.tile_pool(name=f"esm{sfx}", bufs=3) as smpool,
        tc.tile_pool(name=f"ejunk{sfx}", bufs=2) as junkpool,
        tc.tile_pool(name=f"epst{sfx}", bufs=2, space="PSUM") as eps_t,
        tc.tile_pool(name=f"epsz{sfx}", bufs=2, space="PSUM") as eps_z,
        tc.tile_pool(name=f"epso{sfx}", bufs=1, space="PSUM") as eps_o,
    ):
        out_acc = ppool.tile([C, BL], F32)
        nc.vector.memset(out_acc[:, :], 0.0)

        for g in range(NGRP):
            b0 = g * GBL
            # ============ gating for samples [b0, b0+GBL) ============
            # pooled^T accumulated straight into [d-partition, j, sample]
            pts = gspool.tile([128, DT, GBL], F32, tag="pts")
            for bl in range(GBL):
                b = b0 + bl
                gtokT = gpool.tile([128, DT, S], BF16, tag="gtokT")
                nc.gpsimd.dma_gather(
                    out_ap=gtokT[:, :, :],
                    in_ap=emb_t[:, :],
                    idxs_ap=xw[:, b, :],
                    num_idxs=S,
                    num_idxs_reg=S,
                    elem_size=D,
                    transpose=True,
                )
                # sum over s (mean folded into gate_w1 host-side)
                nc.vector.tensor_reduce(
                    out=pts[:, :, bl : bl + 1].rearrange("p j o -> p (j o)"),
                    in_=gtokT[:, :, :],
                    axis=mybir.AxisListType.X,
                    op=mybir.AluOpType.add,
                )

            # gate layer 1 + relu
            hR = gspool.tile([128, MT, GBL], F32, tag="hR")
            for m in range(MT):
                h_ps = gps_s.tile([128, GBL], F32, tag="gmisc")
                for j in range(DT):
                    nc.tensor.matmul(
                        out=h_ps[:, :],
                        lhsT=gw1_sb[:, j, m * 128 : (m + 1) * 128],
                        rhs=pts[:, j, :],
                        start=(j == 0),
                        stop=(j == DT - 1),
                    )
                nc.scalar.activation(
                    out=hR[:, m, :],
                    in_=h_ps[:, :],
                    func=act.Relu,
                    bias=gb1_sb[:, m : m + 1],
                )

            # gate layer 2 -> logits [e, b]
            l_ps = gps_s.tile([E, GBL], F32, tag="gmisc")
            for m in range(MT):
                nc.tensor.matmul(
                    out=l_ps[:, :],
                    lhsT=gw2_sb[:, m, :],
                    rhs=hR[:, m, :],
                    start=(m == 0),
                    stop=(m == MT - 1),
                )
            l_sb = gspool.tile([E, GBL], F32, tag="l_sb")
            nc.scalar.activation(
                out=l_sb[:, :], in_=l_ps[:, :], func=act.Identity,
                bias=gb2_sb[:, 0:1],
            )
            # transpose logits -> [b, e]
            lt_ps = gps_s.tile([GBL, E], F32, tag="gmisc")
            nc.tensor.matmul(
                out=lt_ps[:, :], lhsT=l_sb[:, :], rhs=id_f[0:E, 0:E],
                start=True, stop=True,
            )
            lt_sb = gspool.tile([GBL, E], F32, tag="lt_sb")
            nc.vector.tensor_copy(lt_sb[:, :], lt_ps[:, :])

            # top-2 of logits == top-2 of softmax (monotone)
            mx = gspool.tile([GBL, 8], F32, tag="mx")
            mi = gspool.tile([GBL, 8], U32, tag="mi")
            nc.vector.max_with_indices(mx[:, :], mi[:, :], lt_sb[:, :])

            # renormalized top-2 softmax weights:
            # rw1 = 1/(1+exp(l2-l1)), rw2 = exp(l2-l1)/(1+exp(l2-l1))
            dlt = gspool.tile([GBL, 1], F32, tag="dlt")
            nc.vector.tensor_sub(dlt[:, :], mx[:, 1:2], mx[:, 0:1])
            q = gspool.tile([GBL, 1], F32, tag="q")
            nc.scalar.activation(out=q[:, :], in_=dlt[:, :], func=act.Exp)
            sden = gspool.tile([GBL, 1], F32, tag="sden")
            nc.vector.tensor_scalar_add(sden[:, :], q[:, :], 1.0)
            rw1 = gspool.tile([GBL, 1], F32, tag="rw1")
            nc.vector.reciprocal(rw1[:, :], sden[:, :])
            rw2 = gspool.tile([GBL, 1], F32, tag="rw2")
            nc.vector.tensor_mul(rw2[:, :], q[:, :], rw1[:, :])

            # pack per-(b,k) scalars: cols bl*8 + {0,1}=e*V, {2,3}=e*128,
            # {6,7}=rw ({4,5} unused)
            ei_f = gspool.tile([GBL, TOPK], F32, tag="ei_f")
            nc.vector.tensor_copy(ei_f[:, :], mi[:, 0:TOPK])
            vals = gspool.tile([GBL, 8], F32, tag="vals")
            nc.vector.tensor_scalar_mul(vals[:, 0:2], ei_f[:, :], float(V))
            nc.vector.tensor_scalar_mul(vals[:, 2:4], ei_f[:, :], 128.0)
            nc.vector.tensor_scalar_mul(vals[:, 4:6], ei_f[:, :], 0.0)
            nc.vector.tensor_copy(vals[:, 6:7], rw1[:, :])
            nc.vector.tensor_copy(vals[:, 7:8], rw2[:, :])

            # broadcast across partitions: bounce through DRAM to get a flat
            # [1, GBL*8] row, then K=1 matmul against ones.
            scratch = dpool.tile([GBL, 8], F32, tag=f"scratch{sfx}_{g}")
            nc.sync.dma_start(out=scratch[:, :], in_=vals[:, :])
            if chain is not None:
                # unused col 4: forces rep r to wait on rep r-1's result
                nc.sync.dma_start(out=scratch[0:1, 4:5], in_=chain[0:1, 0:1])
            flat = gspool.tile([1, GBL * 8], F32, tag="flat")
            nc.sync.dma_start(
                out=flat[0:1, :].rearrange("p (b c) -> p b c", b=GBL),
                in_=scratch[:, :],
            )
            bc_ps = gps_s.tile([128, GBL * 8], F32, tag="gmisc")
            nc.tensor.matmul(
                out=bc_ps[:, :], lhsT=ones_m[:, :], rhs=flat[0:1, :],
                start=True, stop=True,
            )
            BCf = bcpool.tile([128, GBL * 8], F32, tag="bcf")
            BCi = bcpool.tile([128, GBL * 8], I32, tag="bci")
            nc.vector.tensor_copy(BCf[:, :], bc_ps[:, :])
            nc.vector.tensor_copy(BCi[:, :], bc_ps[:, :])  # cast f32->i32

            # ============ experts for this group (fp8 DoubleRow) ============
            for bl in range(GBL):
                b = b0 + bl
                for k in range(TOPK):
                    cEV = bl * 8 + k
                    cE128 = bl * 8 + 2 + k
                    cRW = bl * 8 + 6 + k

                    tok_idx = xipool.tile([128, ST], I32, tag="tok_idx")
                    nc.vector.tensor_add(
                        tok_idx[:, :],
                        xt[:, b, :],
                        BCi[:, cEV : cEV + 1].to_broadcast([128, ST]),
                    )
                    w_idx = xipool.tile([128, 1], I32, tag="w_idx")
                    nc.vector.tensor_add(
                        w_idx[:, :], iota_p[:, :], BCi[:, cE128 : cE128 + 1]
                    )

                    # bf16 token rows (pre-scaled x512 on host)
                    tok = tokpool.tile([128, ST, D], BF16, tag="tok")
                    for t in range(ST):
                        nc.gpsimd.indirect_dma_start(
                            out=tok[:, t, :],
                            out_offset=None,
                            in_=eemb_t[:, :],
                            in_offset=IndirectOffsetOnAxis(
                                ap=tok_idx[:, t : t + 1], axis=0
                            ),
                        )
                    # per-expert weights: W1 (fp8, x512) + small bf16 row
                    wg1 = wpool.tile([128, DT * H], F8, tag="wg1")
                    nc.gpsimd.indirect_dma_start(
                        out=wg1[:, :],
                        out_offset=None,
                        in_=wf8_t[:, :],
                        in_offset=IndirectOffsetOnAxis(ap=w_idx[:, :], axis=0),
                    )
                    wgr = wrpool.tile([128, WRCOLS], BF16, tag="wgr")
                    nc.gpsimd.indirect_dma_start(
                        out=wgr[:, :],
                        out_offset=None,
                        in_=wallr_t[:, :],
                        in_offset=IndirectOffsetOnAxis(ap=w_idx[:, :], axis=0),
                    )
                    b1f = smpool.tile([128, HT], F32, tag="b1f")
                    nc.vector.tensor_copy(b1f[:, :], wgr[:, B1COL : B1COL + HT])
                    b2f = smpool.tile([C, 1], F32, tag="b2f")
                    nc.vector.tensor_copy(b2f[:, :], wgr[0:C, B2COL : B2COL + 1])

                    # transpose tok -> tokT[d, s] via matmul against identity,
                    # casting to fp8 in the PSUM->SBUF copy
                    tokT = ttpool.tile([128, DT, S], F8, tag="tokT")
                    for j in range(DT):
                        tp = eps_t.tile([128, S], F32, tag="tp")
                        for t in range(ST):
                            nc.tensor.matmul(
                                out=tp[:, t * 128 : (t + 1) * 128],
                                lhsT=tok[:, t, j * 128 : (j + 1) * 128],
                                rhs=id_bf[:, :],
                                start=True,
                                stop=True,
                            )
                        nc.vector.tensor_copy(tokT[:, j, :], tp[:, :])

                    wview = wg1[:, :].rearrange("p (a h) -> p a h", a=DT)
                    tview = tokT[:, :, :]

                    # z[h_tile] = relu(tokT.T @ W1 + b1*S^2); sum over s.
                    # DoubleRow: 2 k-subtiles per pass, contiguous columns.
                    pacc = smpool.tile([128, HT], F32, tag="pacc")
                    for j2 in range(HT):
                        z_ps = eps_z.tile([128, S], F32, tag="z")
                        for tp2 in range(DT // 2):
                            nc.tensor.matmul(
                                out=z_ps[:, :],
                                lhsT=wview[
                                    :, 2 * tp2 : 2 * tp2 + 2,
                                    j2 * 128 : (j2 + 1) * 128,
                                ],
                                rhs=tview[:, 2 * tp2 : 2 * tp2 + 2, :],
                                start=(tp2 == 0),
                                stop=(tp2 == DT // 2 - 1),
                                perf_mode=mybir.MatmulPerfMode.DoubleRow,
                            )
                        zjunk = junkpool.tile([128, S], BF16, tag="zjunk")
                        nc.scalar.activation(
                            out=zjunk[:, :],
                            in_=z_ps[:, :],
                            func=act.Relu,
                            bias=b1f[:, j2 : j2 + 1],
                            accum_out=pacc[:, j2 : j2 + 1],
                        )

                    # p (bf16) = pacc * 1/(S*SCALE^2); W2 applied hi+lo bf16
                    psc = smpool.tile([128, HT], BF16, tag="psc")
                    nc.vector.tensor_scalar_mul(psc[:, :], pacc[:, :], psc_scale)

                    eo_ps = eps_o.tile([C, 1], F32, tag="eo")
                    for j2 in range(HT):
                        nc.tensor.matmul(
                            out=eo_ps[:, :],
                            lhsT=wgr[:, W2COL + j2 * C : W2COL + (j2 + 1) * C],
                            rhs=psc[:, j2 : j2 + 1],
                            start=(j2 == 0),
                            stop=False,
                        )
                    for j2 in range(HT):
                        nc.tensor.matmul(
                            out=eo_ps[:, :],
                            lhsT=wgr[:, W2LO + j2 * C : W2LO + (j2 + 1) * C],
                            rhs=psc[:, j2 : j2 + 1],
                            start=False,
                            stop=(j2 == HT - 1),
                        )
                    eo1 = smpool.tile([C, 1], F32, tag="eo1")
                    nc.scalar.activation(
                        out=eo1[:, :], in_=eo_ps[:, :], func=act.Identity,
                        bias=b2f[:, 0:1],
                    )
                    eo2 = smpool.tile([C, 1], F32, tag="eo2")
                    nc.vector.tensor_mul(eo2[:, :], eo1[:, :], BCf[0:C, cRW : cRW + 1])
                    nc.vector.tensor_add(
                        out_acc[:, b : b + 1], out_acc[:, b : b + 1], eo2[:, :]
                    )

        if chain is not None:
            nc.vector.tensor_copy(chain[0:1, 0:1], out_acc[0:1, 0:1])
        nc.sync.dma_start(
            out=out_t[:, :].rearrange("b c -> c b"), in_=out_acc[:, :]
        )


def _prep_inputs(inputs):
    """Host-side dtype casts + re-layouts shared by all cores."""
    import ml_dtypes

    f32 = np.float32
    bf16 = ml_dtypes.bfloat16
    f8 = ml_dtypes.float8_e4m3

    x = np.asarray(inputs["x"]).astype(np.int32)
    # int16 indices wrapped for dma_gather: xw16[16g+p, b, c] = x[b, c*16+p]
    xw = x.reshape(B, S // 16, 16).transpose(2, 0, 1).astype(np.int16)  # [16, B, 32]
    xw16 = np.tile(xw, (8, 1, 1))                                       # [128, B, 32]

    emb = np.asarray(inputs["emb"], dtype=f32).astype(bf16)

    # expert embedding rows: bf16, pre-scaled so the device-side fp8 cast
    # (in the transpose PSUM->SBUF copy) lands in e4m3 range
    exp_emb = (
        np.ascontiguousarray(np.asarray(inputs["exp_emb"], dtype=f32).reshape(E * V, D))
        * FP8_SCALE
    ).astype(bf16)

    # W1: fp8 x512, standard t-major layout wf8[e*128+p, t*H+h] = W1[e, t*128+p, h]
    w1 = np.asarray(inputs["exp_w1"], dtype=f32)          # [E, D, H]
    ew1 = w1.reshape(E, DT, 128, H).transpose(0, 2, 1, 3).reshape(E * 128, DT * H)
    wf8 = np.clip(ew1 * FP8_SCALE, -240.0, 240.0).astype(f8)

    w2 = np.asarray(inputs["exp_w2"], dtype=f32)          # [E, H, C]
    ew2 = w2.reshape(E, HT, 128, C).transpose(0, 2, 1, 3).reshape(E * 128, HT * C)
    b1 = np.asarray(inputs["exp_b1"], dtype=f32)          # [E, H]
    b1r = b1.reshape(E, HT, 128).transpose(0, 2, 1).reshape(E * 128, HT)
    b2 = np.asarray(inputs["exp_b2"], dtype=f32)          # [E, C]
    b2slot = np.zeros((E * 128, 1), f32)
    for e in range(E):
        b2slot[e * 128 : e * 128 + C, 0] = b2[e]
    w2hi = ew2.astype(bf16).astype(f32)
    w2lo = ew2 - w2hi
    wallr = np.zeros((E * 128, WRCOLS), f32)
    wallr[:, W2COL : W2COL + HT * C] = w2hi
    wallr[:, W2LO : W2LO + HT * C] = w2lo
    # b1 pre-scaled so relu(z_scaled + b1*S^2) descales via psc_scale
    wallr[:, B1COL : B1COL + HT] = b1r * (FP8_SCALE * FP8_SCALE)
    wallr[:, B2COL : B2COL + 1] = b2slot
    wallr = np.ascontiguousarray(wallr).astype(bf16)

    # mean over S folded into gate_w1
    gw1 = np.ascontiguousarray(np.asarray(inputs["gate_w1"], dtype=f32) / S)
    gb1 = np.ascontiguousarray(
        np.asarray(inputs["gate_b1"], dtype=f32).reshape(MT, 128).T
    )
    gw2 = np.ascontiguousarray(np.asarray(inputs["gate_w2"], dtype=f32))
    gb2 = np.ascontiguousarray(np.asarray(inputs["gate_b2"], dtype=f32).reshape(E, 1))

    shared = dict(
        emb=emb, eemb=exp_emb, wf8=wf8, wallr=wallr,
        gw1=gw1, gb1=gb1, gw2=gw2, gb2=gb2,
    )
    return x, xw16, shared


def kernel(**inputs) -> np.ndarray:
    global last_results
    if "nc" not in _compiled:
        _compiled["nc"] = build_program()
    nc = _compiled["nc"]

    x, xw16, shared = _prep_inputs(inputs)
    in_maps = [
        {
            "x_loc": np.ascontiguousarray(x[c * BL : (c + 1) * BL]),
            "xw16": np.ascontiguousarray(xw16[:, c * BL : (c + 1) * BL]),
            **shared,
        }
        for c in range(NCORES)
    ]
    res = run_bass_kernel_spmd(
        nc, in_maps, list(range(NCORES)),
        trace=os.environ.get("KERNEL_TRACE", "0") == "1",
    )
    last_results = res
    out = np.concatenate([res.results[c]["out"] for c in range(NCORES)], axis=0)
    return np.ascontiguousarray(out.astype(np.float32))
